# revision 1
# baseline (speedup 1.0000x reference)
"""MoE-LoRA Linear kernel for 8 Trainium2 NeuronCores (f32r baseline).

Sharding: core c -> (batch b = c//2, out-feature half = c%2); no
collectives — the router only needs this core's batch.
"""
import sys

sys.path.insert(0, "/opt/trn_rl_repo")

import numpy as np
import ml_dtypes

import concourse.bass as bass
import concourse.mybir as mybir
import concourse.tile as tile
from concourse import bacc, bass_isa
from concourse.bass_utils import run_bass_kernel_spmd

F32 = mybir.dt.float32
F32R = mybir.dt.float32r
BF16 = mybir.dt.bfloat16
BF16_NP = ml_dtypes.bfloat16

D, T, O_SH, E, R = 4096, 2048, 2048, 8, 8
ER = E * R
DT = D // 128
TP = 1024
N_PANEL = T // TP
OT = O_SH // 128
ROUTER_TEMP = 1.0
SCALING = 16.0 / 8.0

_nc_cache = []


def build():
    nc = bacc.Bacc(None, target_bir_lowering=False)
    xT = nc.dram_tensor("xT", [D, T], F32R, kind="ExternalInput")
    Wt = nc.dram_tensor("Wt", [D, O_SH], F32R, kind="ExternalInput")
    At = nc.dram_tensor("At", [D, ER], F32R, kind="ExternalInput")
    Bta = nc.dram_tensor("Bta", [ER + 1, O_SH], F32R, kind="ExternalInput")
    rW = nc.dram_tensor("rW", [D, E], F32R, kind="ExternalInput")
    rb = nc.dram_tensor("rb", [E], F32, kind="ExternalInput")
    ones_d = nc.dram_tensor("ones_d", [T], F32R, kind="ExternalInput")
    out = nc.dram_tensor("out", [O_SH, T], F32, kind="ExternalOutput")
    wscratch = nc.dram_tensor("wscratch", [E], F32)

    with tile.TileContext(nc) as tc:
        with (
            tc.tile_pool(name="xpool", bufs=1) as xpool,
            tc.tile_pool(name="wpool", bufs=2) as wpool,
            tc.tile_pool(name="single", bufs=1) as single,
            tc.tile_pool(name="ev", bufs=2) as evpool,
            tc.tile_pool(name="ps", bufs=2, space="PSUM") as psp,
            tc.tile_pool(name="psmain", bufs=4, space="PSUM") as psm,
            tc.tile_pool(name="ps3", bufs=2, space="PSUM") as ps3,
        ):
            atp = single.tile([128, DT, ER], F32R)
            nc.sync.dma_start(atp[:], At[:].rearrange("(dt p) r -> p dt r", p=128))
            rwp = single.tile([128, DT, E], F32R)
            nc.sync.dma_start(rwp[:], rW[:].rearrange("(dt p) e -> p dt e", p=128))
            bta = single.tile([ER + 1, O_SH], F32R)
            nc.sync.dma_start(bta[:], Bta[:])
            rb8 = single.tile([E, 1], F32)
            nc.sync.dma_start(rb8[:], rb[:, None])
            haug = single.tile([ER + 1, T], F32R)
            nc.sync.dma_start(haug[ER : ER + 1, :], ones_d[None, :])

            xq = single.tile([128, DT, 4 * N_PANEL], F32)

            xpA = xpool.tile([128, DT, 512], F32R, tag="xpA")
            xpB = xpool.tile([128, DT, 512], F32R, tag="xpB")
            xhalves = (xpA, xpB)
            xTr = xT[:].rearrange("(dt p) t -> p dt t", p=128)

            for panel in range(N_PANEL):
                t0 = panel * TP
                for half in range(2):
                    for q in range(2):
                        nc.sync.dma_start(
                            xhalves[half][:, :, q * 256 : (q + 1) * 256],
                            xTr[
                                :, :,
                                t0 + half * 512 + q * 256 : t0 + half * 512 + (q + 1) * 256,
                            ],
                        )
                for tch in range(TP // 512):
                    hps = psp.tile([ER, 512], F32, tag="hps")
                    for d in range(DT):
                        nc.tensor.matmul(
                            hps[:],
                            atp[:, d, :],
                            xhalves[tch][:, d, :],
                            start=(d == 0),
                            stop=(d == DT - 1),
                        )
                    nc.vector.tensor_copy(
                        haug[0:ER, t0 + tch * 512 : t0 + (tch + 1) * 512], hps[:]
                    )
                for q in range(4):
                    nc.vector.reduce_sum(
                        xq[:, :, panel * 4 + q],
                        xhalves[q // 2][:, :, (q % 2) * 256 : (q % 2 + 1) * 256].bitcast(F32),
                        axis=mybir.AxisListType.X,
                    )

                if panel == N_PANEL - 1:
                    xsum_f = single.tile([128, DT], F32)
                    nc.vector.reduce_sum(
                        xsum_f[:], xq[:], axis=mybir.AxisListType.X
                    )
                    lgps = psp.tile([E, 1], F32, tag="hps")
                    for d in range(DT):
                        nc.tensor.matmul(
                            lgps[:],
                            rwp[:, d, :].bitcast(F32),
                            xsum_f[:, d : d + 1],
                            start=(d == 0),
                            stop=(d == DT - 1),
                        )
                    lg8 = single.tile([E, 1], F32)
                    nc.scalar.activation(
                        lg8[:], lgps[:], mybir.ActivationFunctionType.Copy,
                        scale=1.0 / (T * ROUTER_TEMP),
                    )
                    nc.vector.tensor_tensor(lg8[:], lg8[:], rb8[:], mybir.AluOpType.add)
                    m8 = single.tile([E, 1], F32)
                    nc.gpsimd.partition_all_reduce(
                        m8[:], lg8[:], channels=E, reduce_op=bass_isa.ReduceOp.max
                    )
                    e8 = single.tile([E, 1], F32)
                    nc.vector.tensor_tensor(e8[:], lg8[:], m8[:], mybir.AluOpType.subtract)
                    nc.scalar.activation(e8[:], e8[:], mybir.ActivationFunctionType.Exp)
                    s8 = single.tile([E, 1], F32)
                    nc.gpsimd.partition_all_reduce(
                        s8[:], e8[:], channels=E, reduce_op=bass_isa.ReduceOp.add
                    )
                    r8 = single.tile([E, 1], F32)
                    nc.vector.reciprocal(r8[:], s8[:])
                    w8 = single.tile([E, 1], F32)
                    nc.vector.tensor_tensor(w8[:], e8[:], r8[:], mybir.AluOpType.mult)
                    nc.vector.tensor_scalar_mul(w8[:], w8[:], SCALING)
                    nc.sync.dma_start(wscratch[:], w8[:, 0])
                    wexp = single.tile([ER + 1, 1], F32)
                    nc.vector.memset(wexp[ER : ER + 1, :], 1.0)
                    wsrc = bass.AP(tensor=wscratch, offset=0, ap=[[1, E], [0, R]])
                    nc.sync.dma_start(wexp[0:ER, :], wsrc)
                    nc.vector.tensor_tensor(
                        bta[:], bta[:], wexp[:].to_broadcast([ER + 1, O_SH]),
                        mybir.AluOpType.mult,
                    )

                last = panel == N_PANEL - 1
                for o in range(OT):
                    osl = slice(o * 128, (o + 1) * 128)
                    wt = wpool.tile([128, DT, 128], F32R, tag="wt")
                    nc.sync.dma_start(
                        wt[:], Wt[:, osl].rearrange("(dt p) o -> p dt o", p=128)
                    )
                    pstiles = [
                        psm.tile([128, 512], F32, tag="main", name=f"main_{i}")
                        for i in range(2)
                    ]
                    for tch in range(2):
                        for d in range(DT):
                            nc.tensor.matmul(
                                pstiles[tch][:],
                                wt[:, d, :],
                                xhalves[tch][:, d, :],
                                start=(d == 0),
                                stop=(d == DT - 1) and not last,
                            )
                    if last:
                        for tch in range(2):
                            nc.tensor.matmul(
                                pstiles[tch][:],
                                bta[:, osl],
                                haug[:, t0 + tch * 512 : t0 + (tch + 1) * 512],
                                start=False,
                                stop=True,
                            )
                    for tch in range(2):
                        ev = evpool.tile([128, 512], F32, tag="ev")
                        nc.vector.tensor_copy(ev[:], pstiles[tch][:])
                        nc.sync.dma_start(
                            out[osl, t0 + tch * 512 : t0 + (tch + 1) * 512], ev[:]
                        )
                    if last:
                        for tch in range(2):
                            p3 = ps3.tile([128, 512], F32, tag="p3")
                            nc.tensor.matmul(
                                p3[:],
                                bta[:, osl],
                                haug[:, tch * 512 : (tch + 1) * 512],
                                start=True,
                                stop=True,
                            )
                            ev3 = evpool.tile([128, 512], F32, tag="ev")
                            nc.vector.tensor_copy(ev3[:], p3[:])
                            nc.gpsimd.dma_start(
                                out[osl, tch * 512 : (tch + 1) * 512],
                                ev3[:],
                                accum_op=mybir.AluOpType.add,
                            )
    nc.compile()
    return nc


def _get_nc():
    if not _nc_cache:
        _nc_cache.append(build())
    return _nc_cache[0]


def kernel(x, W_base, b_base, lora_A, lora_B, router_W, router_b):
    x = np.asarray(x, dtype=np.float32)
    W_base = np.asarray(W_base, dtype=np.float32)
    b_base = np.asarray(b_base, dtype=np.float32)
    lora_A = np.asarray(lora_A, dtype=np.float32)
    lora_B = np.asarray(lora_B, dtype=np.float32)
    router_W = np.asarray(router_W, dtype=np.float32)
    router_b = np.asarray(router_b, dtype=np.float32)

    B, S, D_ = x.shape
    O = W_base.shape[0]
    At_h = np.ascontiguousarray(lora_A.reshape(E * R, D_).T)
    rW_h = np.ascontiguousarray(router_W.T)
    ones_h = np.ones(T, dtype=np.float32)

    in_maps = []
    for c in range(8):
        b, half = c // 2, c % 2
        osl = slice(half * O_SH, (half + 1) * O_SH)
        Bt = np.ascontiguousarray(
            lora_B[:, osl, :].transpose(0, 2, 1).reshape(E * R, O_SH)
        )
        Bta_h = np.concatenate([Bt, b_base[osl][None, :]], axis=0)
        in_maps.append(
            {
                "xT": np.ascontiguousarray(x[b].T),
                "Wt": np.ascontiguousarray(W_base[osl].T),
                "At": At_h,
                "Bta": np.ascontiguousarray(Bta_h),
                "rW": rW_h,
                "rb": router_b,
                "ones_d": ones_h,
            }
        )

    global _last_in_maps
    _last_in_maps = in_maps
    nc = _get_nc()
    res = run_bass_kernel_spmd(nc, in_maps, core_ids=list(range(8)))
    out = np.empty((B, S, O), dtype=np.float32)
    for c in range(8):
        b, half = c // 2, c % 2
        out[b, :, half * O_SH : (half + 1) * O_SH] = res.results[c]["out"].T
    return out



# revision 2
# speedup vs baseline: 1.2091x; 1.2091x over previous
"""MoE-LoRA Linear kernel for 8 Trainium2 NeuronCores (bf16).

Sharding: core c -> (batch b = c//2, out-feature half = c%2); no
collectives — the router only needs this core's batch.

Structure: x (bf16, 16MB) is fully SBUF-resident, loaded once in 4
token chunks. LoRA-h matmuls free-ride the router logits (router_W
appended as extra stationary columns). The softmax sum is computed on
the tensor engine with an all-ones stationary (sums partitions 64-71
and broadcasts to partitions 0-63 in one matmul), keeping the gpsimd
queue free as a dedicated W-tile DMA channel. The router-weighted
LoRA + bias rows are folded into every o-tile's PSUM accumulation, so
there is no replay pass and no read-modify-write output DMA.
"""
import sys

sys.path.insert(0, "/opt/trn_rl_repo")

import numpy as np
import ml_dtypes

import concourse.bass as bass
import concourse.mybir as mybir
import concourse.tile as tile
from concourse import bacc
from concourse.bass_utils import run_bass_kernel_spmd

F32 = mybir.dt.float32
BF16 = mybir.dt.bfloat16
BF16_NP = ml_dtypes.bfloat16

D, T, O_SH, E, R = 4096, 2048, 2048, 8, 8
ER = E * R          # 64 lora rows
ERE = ER + E        # 72 = lora rows + router logit rows
DT = D // 128       # 32 contraction chunks
TCH = T // 512      # 4 token chunks of 512
OT = O_SH // 128    # 16 out-feature tiles
ROUTER_TEMP = 1.0
SCALING = 16.0 / 8.0

_nc_cache = []


def build():
    nc = bacc.Bacc(None, target_bir_lowering=False)
    xh = nc.dram_tensor("xh", [TCH, 128, DT, 512], BF16, kind="ExternalInput")
    Wts = nc.dram_tensor("Wts", [OT, 128, DT, 128], BF16, kind="ExternalInput")
    atp = nc.dram_tensor("atp", [128, DT, ERE], BF16, kind="ExternalInput")
    bta = nc.dram_tensor("bta", [ER + 1, O_SH], BF16, kind="ExternalInput")
    rb = nc.dram_tensor("rb", [E], F32, kind="ExternalInput")
    out = nc.dram_tensor("out", [O_SH, T], F32, kind="ExternalOutput")
    wscratch = nc.dram_tensor("wscratch", [E], F32)

    with tile.TileContext(nc) as tc:
        with (
            tc.tile_pool(name="big", bufs=1) as big,
            tc.tile_pool(name="wpool", bufs=3) as wpool,
            tc.tile_pool(name="ev", bufs=4) as evpool,
            tc.tile_pool(name="psm", bufs=6, space="PSUM") as psm,
            tc.tile_pool(name="psh", bufs=2, space="PSUM") as psh,
        ):
            # ---- load order on sync queue: atp, x chunks, bta, rb ----
            atp_t = big.tile([128, DT, ERE], BF16, tag="atp")
            nc.sync.dma_start(atp_t[:], atp[:])
            xsb = []
            for c in range(TCH):
                xt = big.tile([128, DT, 512], BF16, tag=f"x{c}")
                nc.sync.dma_start(xt[:], xh[c])
                xsb.append(xt)
            bta_t = big.tile([ER + 1, O_SH], BF16, tag="bta")
            nc.sync.dma_start(bta_t[:], bta[:])
            rbt = big.tile([ERE, 1], F32, tag="rbt")
            nc.sync.dma_start(rbt[ER:ERE, :], rb[:, None])

            haug = big.tile([ER + 1, T], BF16, tag="haug")
            nc.vector.memset(haug[ER : ER + 1, :], 1.0)
            ones_t = big.tile([ERE, ER], BF16, tag="ones")
            nc.vector.memset(ones_t[ER:ERE, :], 1.0)
            lg = big.tile([ERE, T], F32, tag="lg")

            # ---- gpsimd queue: dedicated W-tile DMA channel ----
            def load_wt(o):
                wt = wpool.tile([128, DT, 128], BF16, tag="wt", name=f"wt{o}")
                nc.gpsimd.dma_start(wt[:], Wts[o])
                return wt

            wt0 = load_wt(0)

            # ---- o-tile 0 interleaved with h chains per x chunk ----
            ps0 = []
            for c in range(TCH):
                csl = slice(c * 512, (c + 1) * 512)
                hps = psh.tile([ERE, 512], F32, tag="hps", name=f"h{c}")
                for d in range(DT):
                    nc.tensor.matmul(
                        hps[:],
                        atp_t[:, d, :],
                        xsb[c][:, d, :],
                        start=(d == 0),
                        stop=(d == DT - 1),
                    )
                nc.vector.tensor_copy(haug[0:ER, csl], hps[0:ER, :])
                nc.vector.tensor_copy(lg[ER:ERE, csl], hps[ER:ERE, :])
                p = psm.tile([128, 512], F32, tag="main", name=f"m0_{c}")
                for d in range(DT):
                    nc.tensor.matmul(
                        p[:],
                        wt0[:, d, :],
                        xsb[c][:, d, :],
                        start=(d == 0),
                        stop=False,
                    )
                ps0.append(p)

            # ---- router: logits -> softmax -> scale haug rows ----
            lgr = big.tile([ERE, 1], F32, tag="lgr")
            nc.vector.reduce_sum(lgr[ER:ERE, :], lg[ER:ERE, :], axis=mybir.AxisListType.X)
            lg8 = big.tile([ERE, 1], F32, tag="lg8")
            nc.scalar.activation(
                lg8[ER:ERE, :], lgr[ER:ERE, :], mybir.ActivationFunctionType.Copy,
                scale=1.0 / (T * ROUTER_TEMP),
            )
            nc.vector.tensor_tensor(
                lg8[ER:ERE, :], lg8[ER:ERE, :], rbt[ER:ERE, :], mybir.AluOpType.add
            )
            # logits here are tiny (|l| < ~0.2): exp without max-subtraction.
            e8f = big.tile([ERE, 1], F32, tag="e8f")
            nc.scalar.activation(e8f[ER:ERE, :], lg8[ER:ERE, :], mybir.ActivationFunctionType.Exp)
            e8b = big.tile([ERE, 1], BF16, tag="e8b")
            nc.vector.tensor_copy(e8b[ER:ERE, :], e8f[ER:ERE, :])
            # sum exp over the 8 experts AND broadcast to partitions 0-63
            # in one matmul: ones[8,64].T @ e8[8,1] -> [64,1].
            sps = psh.tile([ER, 1], F32, tag="hps", name="sps")
            nc.tensor.matmul(
                sps[:], ones_t[ER:ERE, :], e8b[ER:ERE, :], start=True, stop=True
            )
            ssb = big.tile([ER, 1], F32, tag="ssb")
            nc.vector.tensor_copy(ssb[:], sps[:])
            rsb = big.tile([ER, 1], F32, tag="rsb")
            nc.vector.reciprocal(rsb[:], ssb[:])
            # replicate the 8 raw exps to 64 rows via a dram round trip
            nc.sync.dma_start(wscratch[:], e8f[ER:ERE, 0])
            wexpf = big.tile([ER, 1], F32, tag="wexpf")
            wsrc = bass.AP(tensor=wscratch, offset=0, ap=[[1, E], [0, R]])
            nc.sync.dma_start(wexpf[:], wsrc)
            wexpn = big.tile([ER, 1], F32, tag="wexpn")
            nc.vector.tensor_tensor(wexpn[:], wexpf[:], rsb[:], mybir.AluOpType.mult)
            wexpb = big.tile([ER, 1], BF16, tag="wexpb")
            nc.vector.tensor_copy(wexpb[:], wexpn[:])
            nc.vector.tensor_tensor(
                haug[0:ER, :], haug[0:ER, :], wexpb[:].to_broadcast([ER, T]),
                mybir.AluOpType.mult,
            )

            def chain(o, wt, c, name):
                p = psm.tile([128, 512], F32, tag="main", name=name)
                for d in range(DT):
                    nc.tensor.matmul(
                        p[:],
                        wt[:, d, :],
                        xsb[c][:, d, :],
                        start=(d == 0),
                        stop=False,
                    )
                return p

            def aug(o, pstiles):
                osl = slice(o * 128, (o + 1) * 128)
                for c in range(TCH):
                    nc.tensor.matmul(
                        pstiles[c][:],
                        bta_t[:, osl],
                        haug[:, c * 512 : (c + 1) * 512],
                        start=False,
                        stop=True,
                    )

            def evict(o, pstiles):
                osl = slice(o * 128, (o + 1) * 128)
                for c in range(TCH):
                    ev = evpool.tile([128, 512], F32, tag="ev")
                    nc.vector.tensor_copy(ev[:], pstiles[c][:])
                    nc.scalar.dma_start(out[osl, c * 512 : (c + 1) * 512], ev[:])

            # ---- o-tile 1: first two chains before aug(o0) frees banks ----
            wt1 = load_wt(1)
            ps1 = [chain(1, wt1, c, f"m1_{c}") for c in (0, 1)]
            aug(0, ps0)
            evict(0, ps0)
            ps1 += [chain(1, wt1, c, f"m1_{c}") for c in (2, 3)]
            aug(1, ps1)
            evict(1, ps1)

            # ---- o-tiles 2..15 ----
            for o in range(2, OT):
                wt = load_wt(o)
                ps = [chain(o, wt, c, f"m{o}_{c}") for c in range(TCH)]
                aug(o, ps)
                evict(o, ps)

    nc.compile()
    return nc


def _get_nc():
    if not _nc_cache:
        _nc_cache.append(build())
    return _nc_cache[0]


def kernel(x, W_base, b_base, lora_A, lora_B, router_W, router_b):
    x = np.asarray(x, dtype=np.float32)
    W_base = np.asarray(W_base, dtype=np.float32)
    b_base = np.asarray(b_base, dtype=np.float32)
    lora_A = np.asarray(lora_A, dtype=np.float32)
    lora_B = np.asarray(lora_B, dtype=np.float32)
    router_W = np.asarray(router_W, dtype=np.float32)
    router_b = np.asarray(router_b, dtype=np.float32)

    B, S, D_ = x.shape
    O = W_base.shape[0]

    M = np.concatenate([lora_A.reshape(ER, D_), router_W], axis=0)  # [72, D]
    atp_h = np.ascontiguousarray(
        M.reshape(ERE, DT, 128).transpose(2, 1, 0).astype(BF16_NP)
    )
    rb_h = router_b.astype(np.float32)

    xh_b = []
    for b in range(B):
        xh_b.append(
            np.ascontiguousarray(
                x[b].reshape(TCH, 512, DT, 128).transpose(0, 3, 2, 1).astype(BF16_NP)
            )
        )
    Wts_h, bta_h = [], []
    for half in range(2):
        osl = slice(half * O_SH, (half + 1) * O_SH)
        Wts_h.append(
            np.ascontiguousarray(
                W_base[osl].reshape(OT, 128, DT, 128).transpose(0, 3, 2, 1).astype(BF16_NP)
            )
        )
        bta_lo = SCALING * lora_B[:, osl, :].transpose(0, 2, 1).reshape(ER, O_SH)
        bta_h.append(
            np.ascontiguousarray(
                np.concatenate([bta_lo, b_base[osl][None, :]], axis=0).astype(BF16_NP)
            )
        )

    in_maps = []
    for c in range(8):
        b, half = c // 2, c % 2
        in_maps.append(
            {
                "xh": xh_b[b],
                "Wts": Wts_h[half],
                "atp": atp_h,
                "bta": bta_h[half],
                "rb": rb_h,
            }
        )

    global _last_in_maps
    _last_in_maps = in_maps
    nc = _get_nc()
    res = run_bass_kernel_spmd(nc, in_maps, core_ids=list(range(8)))
    out = np.empty((B, S, O), dtype=np.float32)
    for c in range(8):
        b, half = c // 2, c % 2
        out[b, :, half * O_SH : (half + 1) * O_SH] = res.results[c]["out"].T
    return out
